# revision 22
# baseline (speedup 1.0000x reference)
"""GAT layer (nn_GATLayerAdj) Trainium2 Bass kernel, 8-core SPMD — v2.

Reference computation (N=1024, di=do=64):
    a[i,j]  = x[j]@w_src + x[i]@w_tgt + bw        (attention logits)
    att     = softmax_j(where(adj>0, a, -1e16))
    y[i,j,:]= relu(x[j]@WfS.T + x[i]@WfT.T + bf)
    o[i,:]  = sum_j att[i,j] * y[i,j,:]

Sharding: target-node dim i split across 8 cores (128 rows each).

Key algebraic restructurings vs v1 (which ran ~74us):
  1. Row-softmax is invariant to the per-row shift a_tgt[i]+bw, so the
     attention weights reduce to eT[j,i] = exp(a_src[j]) * adjT[j,i] —
     one per-partition ACT scale-copy per j-chunk in TRANSPOSED layout
     (j on partitions). No logits outer product, no big exp, no PE
     transposes, no identity matrix.
  2. relu(ys_j + u_i) = max(ys_j, -u_i) + u_i. The elementwise build
     becomes ONE tensor_tensor max per half-chunk (bf16 2x mode) —
     v1 needed an add (DVE 2x) plus a relu (ACT/DVE) per half. The
     +u_i correction is exact and cheap: o_i = (sum_j eT*M)/s_i + u_i,
     applied at evacuation by a fused DVE scalar_tensor_tensor
     (t_acc * (1/s) - nurep2) over a diagonal-replicated -u tile built
     with 4 partition-broadcast DMAs.
  3. A few max half-chunks run on the otherwise-idle Pool (gpsimd)
     engine (slower per element, but parallel to the DVE).

Per-core pipeline:
  - PE smalls: nu = -(xb@WfT.T+bf) (negated weights from host), ys
    chunks, a_src row; ACT: exp(a_src) -> es; DRAM round trips
    broadcast nu across partitions (nurep, split over 2 DMA queues)
    and re-layout es to a per-partition column (es_col).
  - eT chunks: ACT Copy(adjT_chunk, scale=es_col[:,c]); row sums s via
    8 accumulating PE matmuls (lhsT=eT chunk, rhs=ones); 1/s on DVE.
  - Per chunk: M = max(ys_bcast, nurep) into r_c (DVE or Pool per
    half); reduce t_acc[i,(i',d)] += eT_chunk^T @ r_c as 4x4
    col-tiled PE matmuls (tile_position groups run concurrently).
  - Tail: t_sb = t_acc*(1/s) - nurep2 (DVE, fused), DMA out; host
    gathers the 32-wide diagonal (pure indexing).

Numerics: bf16 inputs, fp32 PSUM accumulation. The max keeps one
operand exact; u enters through the same bf16 rounding as v1.
"""

from contextlib import ExitStack

import numpy as np
import ml_dtypes

import concourse.bass as bass
import concourse.tile as tile
from concourse import bacc, mybir
from concourse.bass_utils import run_bass_kernel_spmd

# Lighter TileContext exit: stock emits drain + full butterfly barrier +
# sem clears + second butterfly (~11us). Engines already sync at program
# end; keep the drain (output DMA completion), a sem-only rendezvous
# before the clears, and drop the trailing barrier.
import concourse.tile as _tile_mod

if not getattr(_tile_mod, "_exit_trimmed", False):
    def _drain_and_barrier_trim(self, tick_clock, wait_clock):
        from concourse.tile import ScopedClock
        nc = self.nc
        drain_inst = nc.sync.drain()
        wait_clock.add_sem_waits(
            drain_inst.ins, ScopedClock({None: tick_clock.global_clock})
        )
        # parallel rendezvous: every engine incs one sem; gpsimd waits,
        # clears the tile sems, and the program ends (engines sync at
        # program completion anyway - no trailing butterfly needed)
        exit_sem = nc.alloc_semaphore("exit_rdv")
        for eng in (nc.sync, nc.tensor, nc.vector, nc.scalar):
            eng.nop(nofuse=True).then_inc(exit_sem, 1)
        nc.gpsimd.wait_ge(exit_sem, 4)
        assert self.sems is not None
        popped = nc._tile_sem_poison_stack.pop()
        assert popped is self._sem_poison
        nc.clear_and_free_semaphores(list(self.sems.allocated().values()))
        nc.gpsimd.sem_clear(range(exit_sem.num, exit_sem.num + 1))

    _tile_mod.TileContext._drain_and_barrier = _drain_and_barrier_trim
    _tile_mod._exit_trimmed = True

N = 1024
DI = 64
DO = 64
N_CORES = 8
ROWS = N // N_CORES          # 128 target rows per core
NCHUNK = N // 128            # 8 j-chunks
F_FULL = ROWS * DO           # 8192 free size of (i, d)
HALF = F_FULL // 2           # 4096: half-chunk unit

f32 = mybir.dt.float32
bf16 = mybir.dt.bfloat16
AF = mybir.ActivationFunctionType
ALU = mybir.AluOpType

# (chunk, half) pairs whose max runs on the Pool engine instead of DVE.
# (TensorTensor max is NOT a valid Pool opcode on TRN2 — keep empty.)
POOL_HALVES = frozenset()

_CACHE = {}


def _build_program():
    nc = bacc.Bacc("TRN2", target_bir_lowering=False, debug=False,
                   num_devices=N_CORES)

    # ---- DRAM I/O ----
    xT_d = nc.dram_tensor("xT", [DI, N], bf16, kind="ExternalInput").ap()
    wfsT_d = nc.dram_tensor("wfsT", [DI, DO], bf16, kind="ExternalInput").ap()
    ws_d = nc.dram_tensor("ws", [DI, 1], bf16, kind="ExternalInput").ap()
    nwfta_d = nc.dram_tensor("nwfta", [DI + 1, DO], bf16, kind="ExternalInput").ap()
    xbTa_d = nc.dram_tensor("xbTa", [DI + 1, ROWS], bf16, kind="ExternalInput").ap()
    adjT_d = nc.dram_tensor("adjT", [ROWS, N], bf16, kind="ExternalInput").ap()
    o_d = nc.dram_tensor("o", [128, 2048], f32, kind="ExternalOutput").ap()

    with tile.TileContext(nc) as tc, ExitStack() as ctx:
        cons = ctx.enter_context(tc.tile_pool(name="cons", bufs=1))
        rp = ctx.enter_context(tc.tile_pool(name="rp", bufs=3))
        psp = ctx.enter_context(tc.tile_pool(name="psp", bufs=3, space="PSUM"))
        accs = ctx.enter_context(tc.tile_pool(name="accs", bufs=1, space="PSUM"))
        accp = ctx.enter_context(tc.tile_pool(name="accp", bufs=1, space="PSUM"))

        # ---- input DMAs (u-chain inputs first: longest dep chain) ----
        xbTa_t = cons.tile([DI + 1, ROWS], bf16)
        nc.sync.dma_start(xbTa_t[:], xbTa_d[:, :])
        nwfta_t = cons.tile([DI + 1, DO], bf16)
        nc.sync.dma_start(nwfta_t[:], nwfta_d[:, :])
        xT_t = cons.tile([DI, N], bf16)
        nc.sync.dma_start(xT_t[:], xT_d[:, :])
        wfsT_t = cons.tile([DI, DO], bf16)
        nc.sync.dma_start(wfsT_t[:], wfsT_d[:, :])
        ws_t = cons.tile([DI, 1], bf16)
        nc.sync.dma_start(ws_t[:], ws_d[:, :])
        adjT_t = cons.tile([ROWS, N], bf16)
        nc.scalar.dma_start(adjT_t[:], adjT_d[:, :])

        # ---- nu = -(xb@WfT.T + bf)  [128, 64] (K=65 ones-row trick) ----
        nu_ps = psp.tile([ROWS, DO], f32, tag="pre")
        nc.tensor.matmul(nu_ps[:], xbTa_t[:], nwfta_t[:], start=True, stop=True)
        nu_sb = cons.tile([ROWS, DO], bf16)
        nc.scalar.copy(nu_sb[:], nu_ps[:])
        # stage flat to DRAM, then partition-broadcast reads
        nu_dram = nc.dram_tensor("nu_stage", [F_FULL], bf16).ap()
        nc.sync.dma_start(out=nu_dram.rearrange("(i d) -> i d", i=ROWS),
                          in_=nu_sb[:, :])
        nurep = cons.tile([128, F_FULL], bf16)
        for h, eng in ((0, nc.sync), (1, nc.scalar)):
            sl = slice(HALF * h, HALF * (h + 1))
            src = nu_dram[sl]
            bsrc = bass.AP(tensor=src.tensor, offset=src.offset,
                           ap=[[0, 128]] + [list(d) for d in src.ap])
            eng.dma_start(out=nurep[:, sl], in_=bsrc)
        # diagonal-replicated -u for the evacuation fix-up:
        # nurep2[p, f] = nu[32*(p//32) + f//64, f%64]
        nurep2 = cons.tile([128, 2048], bf16)
        for b in range(4):
            src = nu_dram[2048 * b:2048 * (b + 1)]
            bsrc = bass.AP(tensor=src.tensor, offset=src.offset,
                           ap=[[0, 32]] + [list(d) for d in src.ap])
            nc.gpsimd.dma_start(out=nurep2[32 * b:32 * (b + 1), :], in_=bsrc)

        # ---- a_src row + exp -> es, re-laid out per-partition ----
        es_row = cons.tile([1, N], bf16)
        for h in range(2):
            hs = slice(512 * h, 512 * (h + 1))
            asp = psp.tile([1, 512], f32, tag="pre", name=f"asp{h}")
            nc.tensor.matmul(asp[:], ws_t[:], xT_t[:, hs], start=True, stop=True)
            nc.scalar.activation(es_row[:, hs], asp[:], AF.Exp)
        es_dram = nc.dram_tensor("es_stage", [N], bf16).ap()
        nc.gpsimd.dma_start(out=es_dram.rearrange("(o f) -> o f", o=1),
                            in_=es_row[:, :])
        # f32: ACT scale APs must be FP32; the gpsimd (SWDGE) DMA casts
        es_col = cons.tile([128, NCHUNK], f32)
        nc.gpsimd.dma_start(out=es_col[:, :],
                            in_=es_dram.rearrange("(c p) -> p c", p=128))

        # ---- ys chunks: ys_jp[j_local, 64*c + d] = ys[128*c + j_local, d] ----
        ys_jp = cons.tile([128, NCHUNK * DO], bf16)
        for c in range(NCHUNK):
            ysp = psp.tile([128, DO], f32, tag="pre", name=f"ysp{c}")
            nc.tensor.matmul(ysp[:], xT_t[:, 128 * c:128 * (c + 1)], wfsT_t[:],
                             start=True, stop=True)
            nc.scalar.copy(ys_jp[:, DO * c:DO * (c + 1)], ysp[:])

        # ---- eT chunks (ACT scale-copy) + row sums s (PE accum) ----
        onescol = cons.tile([128, 1], bf16)
        nc.vector.memset(onescol[:], 1.0)
        et_all = cons.tile([128, N], bf16)
        ssum_ps = accs.tile([ROWS, 1], f32, tag="acc")
        for c in range(NCHUNK):
            cs = slice(128 * c, 128 * (c + 1))
            nc.scalar.activation(et_all[:, cs], adjT_t[:, cs], AF.Copy,
                                 bias=0.0, scale=es_col[:, c:c + 1])
            nc.tensor.matmul(ssum_ps[:], et_all[:, cs], onescol[:],
                             start=(c == 0), stop=(c == NCHUNK - 1),
                             skip_group_check=True)
        r_t = cons.tile([ROWS, 1], f32)

        # ---- max build + reduce, software-pipelined ----
        SKEW = 2
        t_acc = accp.tile([128, 2048], f32, tag="acc")
        r_tiles = {}

        def emit_build(c):
            r_c = rp.tile([128, F_FULL], bf16, name="r_c")
            r_tiles[c] = r_c
            ys_c = ys_jp[:, DO * c:DO * (c + 1)]
            ys_b = ys_c.rearrange("p d -> p () d").broadcast_to(
                (128, HALF // DO, DO))
            for h in range(2):
                sl = slice(HALF * h, HALF * (h + 1))
                rv = r_c[:, sl].rearrange("p (i d) -> p i d", i=HALF // DO)
                nuv = nurep[:, sl].rearrange("p (i d) -> p i d", i=HALF // DO)
                eng = nc.gpsimd if (c, h) in POOL_HALVES else nc.vector
                eng.tensor_tensor(rv, ys_b, nuv, ALU.max)

        def emit_reduce(c):
            r_c = r_tiles.pop(c)
            for n2 in range(4):
                for b in range(4):
                    nc.tensor.matmul(
                        t_acc[32 * b:32 * (b + 1), 512 * n2:512 * (n2 + 1)],
                        et_all[:, 128 * c + 32 * b:128 * c + 32 * (b + 1)],
                        r_c[:, 2048 * b + 512 * n2:2048 * b + 512 * (n2 + 1)],
                        start=(c == 0),
                        stop=(c == NCHUNK - 1),
                        skip_group_check=True,
                        tile_position=(0, 32 * b),
                    )

        for cc in range(NCHUNK + SKEW):
            if cc < NCHUNK:
                emit_build(cc)
            if cc == 3:
                # mid-queue on the DVE: ssum is long done by now
                nc.vector.reciprocal(r_t[:], ssum_ps[:])
            if cc >= SKEW:
                emit_reduce(cc - SKEW)

        # ---- tail: o = t_acc*(1/s) - nurep2, fused on DVE; DMA out ----
        t_sb = cons.tile([128, 2048], f32)
        for n2 in range(4):
            sl = slice(512 * n2, 512 * (n2 + 1))
            nc.vector.scalar_tensor_tensor(
                t_sb[:, sl], t_acc[:, sl], r_t[:], nurep2[:, sl],
                ALU.mult, ALU.subtract)
            eng = nc.sync if n2 % 2 == 0 else nc.scalar
            eng.dma_start(o_d[:, sl], t_sb[:, sl])

    nc.compile()
    return nc


def _prep_inputs(x, adj, Wf, bf_, Ww, bw):
    b = ml_dtypes.bfloat16
    xT = np.ascontiguousarray(x.T).astype(b)                         # [64, N]
    wfsT = np.ascontiguousarray(Wf[:, :DI].T).astype(b)              # [64, 64]
    ws = np.ascontiguousarray(Ww[0, :DI].reshape(DI, 1)).astype(b)   # [64, 1]
    nwfta = (-np.vstack([Wf[:, DI:].T, bf_[None, :]])).astype(b)     # [65, 64]

    shared = dict(xT=xT, wfsT=wfsT, ws=ws, nwfta=nwfta)
    in_maps = []
    for c in range(N_CORES):
        blk = slice(ROWS * c, ROWS * (c + 1))
        xbTa = np.vstack([x[blk].T, np.ones((1, ROWS), np.float32)])
        # adjT chunk-major: adjT[j_loc, 128c + i] = adj[blk0+i, 128c+j_loc]
        adjT = (adj[blk].T.reshape(NCHUNK, 128, ROWS)
                .transpose(1, 0, 2).reshape(128, N))
        m = dict(shared)
        m["xbTa"] = np.ascontiguousarray(xbTa).astype(b)
        m["adjT"] = np.ascontiguousarray(adjT).astype(b)
        in_maps.append(m)
    return in_maps


def get_program():
    if "nc" not in _CACHE:
        _CACHE["nc"] = _build_program()
    return _CACHE["nc"]


def kernel(x, adj, Wf, bf, Ww, bw):
    x = np.asarray(x, dtype=np.float32)
    adj = np.asarray(adj, dtype=np.int32)
    Wf = np.asarray(Wf, dtype=np.float32)
    bf_ = np.asarray(bf, dtype=np.float32)
    Ww = np.asarray(Ww, dtype=np.float32)
    bw = np.asarray(bw, dtype=np.float32)
    assert x.shape == (N, DI) and adj.shape == (N, N)

    nc = get_program()
    in_maps = _prep_inputs(x, adj, Wf, bf_, Ww, bw)
    res = run_bass_kernel_spmd(nc, in_maps, core_ids=list(range(N_CORES)))
    p_idx = np.arange(128)
    col0 = (p_idx % 32) * DO
    out = np.empty((N, DO), np.float32)
    for c in range(N_CORES):
        t = res.results[c]["o"]                      # [128, 2048]
        out[ROWS * c:ROWS * (c + 1)] = t[p_idx[:, None],
                                         col0[:, None] + np.arange(DO)[None, :]]
    return out
